# revision 27
# baseline (speedup 1.0000x reference)
"""GATv2 layer kernel for Trainium2 (8 NeuronCores, SPMD).

Math note: in the reference, the per-edge value vectors are gathered from the
*destination* node (Vv = V[dest] @ Wv^T + bv) and the scatter-softmax is also
grouped by destination. Within a destination segment Vv is constant, and the
softmax weights sum to 1, so

    H[n] = (V[n] @ Wv_w^T + Wv_b) * [n has >= 1 incoming edge]

exactly (up to f32 rounding of the softmax-weight sum, ~1e-7 relative).

Sharding: nodes are partitioned contiguously across the 8 cores; the small
[128,128] weight is replicated. Each core computes the Wv projection of its
node shard.

Fast path (every node has >= 1 incoming edge — true with overwhelming
probability at E/N = 12.5, and checked on host): the projection is computed
transposed, hT = Wv @ V_shard^T, with the weight as the PE's stationary
operand and 512-node column blocks of V^T streamed through as the moving
operand. I/O is fp16 (inputs quantized on host, output upcast on host),
halving HBM traffic; bias-add rides the PSUM->SBUF cast on the DVE as a
per-partition scalar. Well within the 2e-2 tolerance (~1e-3 max rel err).

Fallback (some node uncovered): the original f32 masked variant — node-tiled
matmuls plus a GPSIMD SWDGE scatter-add histogram over local edge
destinations to build the incoming-edge mask on-device.
"""

import numpy as np

import concourse.bacc as bacc
import concourse.bass as bass
import concourse.mybir as mybir
import concourse.tile as tile
from concourse.bass_utils import run_bass_kernel_spmd
from concourse.library_config import mlp

N_CORES = 8
P = 128
D = 128
NB = 512  # node block: one PSUM bank of f32, max f32 moving free-dim
TABLE_W = 64  # f32 words per histogram-table row -> 256B stride (SWDGE req.)

_module_cache = {}

# Cap indices per SWDGE scatter-add: the Q7 expands indices to int32 in
# local scratch (4096 validated on HW; 8192 crashes the exec unit).
MAX_IDXS_PER_SCATTER = 4096


def _chunking(pad_idx):
    n_chunks = -(-pad_idx // MAX_IDXS_PER_SCATTER)
    per_chunk = -(-pad_idx // (n_chunks * P)) * P
    return n_chunks, per_chunk


def _even_split(n, k):
    """Split n items into k contiguous runs, sizes as even as possible."""
    base, rem = divmod(n, k)
    runs = []
    start = 0
    for i in range(k):
        sz = base + (1 if i < rem else 0)
        runs.append((start, sz))
        start += sz
    return [r for r in runs if r[1] > 0]


def _build_module_fast(NP):
    """Maskless SPMD program: hT = wvT.T @ vT + b, fp16 in/out.

    NP: padded per-core node count (multiple of 128). vT/hT are [D, NP].
    Loads ride the Sync HWDGE ring, stores the Scalar (ACT) ring, so input
    and output streams issue in parallel; the PSUM->SBUF bias-add copies
    alternate between Vector and GpSimd.
    """
    import os

    f32 = mybir.dt.float32
    f16 = mybir.dt.float16

    psbufs = int(os.environ.get("K_PSBUFS", "6"))
    repeat = int(os.environ.get("K_REPEAT", "1"))  # timing experiments only
    n_warm = int(os.environ.get("K_NWARM", "0"))
    # per-load-chunk / per-store-group issuing engines. L0 gets the Sync
    # HWDGE ring to itself so its completion isn't delayed by packet
    # round-robin with later chunks; remaining loads split between the ACT
    # HWDGE ring and GPSIMD's SWDGE queues; stores reuse the Sync ring,
    # which is idle again by the time the first store issues.
    # all loads on ONE HWDGE ring: blocks arrive strictly in order (the PE
    # consumes them in order, so out-of-order arrival head-of-line blocks)
    # and the ring drains at full SDMA attention. Stores stream concurrently
    # on GPSIMD's SWDGE queue row; the last store takes the ACT HWDGE ring
    # for its faster completion receipt.
    load_engs = os.environ.get("K_LENG", "sync").split(",")
    store_engs = os.environ.get(
        "K_SENG", "gpsimd,gpsimd,gpsimd,gpsimd,gpsimd,scalar").split(",")
    pair_copy = os.environ.get("K_PAIRCOPY", "0") == "1"

    # node blocks: one f32 PSUM bank each (= max f32 moving free-dim)
    blocks = []
    col = 0
    while col < NP:
        w = min(NB, NP - col)
        blocks.append((col, w))
        col += w
    n_blocks = len(blocks)

    def _runs(spec_env, default):
        spec = [int(x) for x in os.environ.get(spec_env, default).split(",")]
        runs = []
        b = 0
        for sz in spec:
            if b >= n_blocks:
                break
            sz = min(sz, n_blocks - b)
            runs.append((b, sz))
            b += sz
        while b < n_blocks:  # spill: repeat last size
            sz = min(spec[-1], n_blocks - b)
            runs.append((b, sz))
            b += sz
        return runs

    # first load chunks small -> compute starts early and never stalls on
    # a big chunk; first store small -> write stream overlaps read stream;
    # last store small -> short drain tail
    load_runs = _runs("K_LOADS", "1,2,2,4,2,1,1")
    store_runs = _runs("K_STORES", "2,4,4,2,1")
    max_load_nb = max(nb for _, nb in load_runs)
    max_store_nb = max(nb for _, nb in store_runs)

    nc = bacc.Bacc("TRN2", target_bir_lowering=False, debug=False)
    vT_in = nc.dram_tensor("vT", [D, NP], f16, kind="ExternalInput")
    wvT_in = nc.dram_tensor("wvT", [D, D], f16, kind="ExternalInput")
    bT_in = nc.dram_tensor("bT", [D, 1], f32, kind="ExternalInput")
    hT_out = nc.dram_tensor("hT", [D, NP], f16, kind="ExternalOutput")

    with tile.TileContext(nc) as tc:
        with (
            tc.tile_pool(name="const", bufs=1) as cpool,
            tc.tile_pool(name="vg", bufs=len(load_runs) + 1) as vpool,
            tc.tile_pool(name="hg", bufs=len(store_runs) + 1) as hpool,
            tc.tile_pool(name="psh", bufs=psbufs, space="PSUM") as pspool,
        ):
            wv_sb = cpool.tile([D, D], f16)
            bT_sb = cpool.tile([D, 1], f32)

            # HAM warm-up: ~4us of back-to-back dummy matmuls bridges the
            # PE from the engine-start barrier to the first real matmul so
            # the 2.4 GHz clock-gate is open (and stays open; the real
            # matmul cadence never idles past the ~3.4us MID window).
            if n_warm:
                warm_sb = cpool.tile([P, P], f16)
                nc.vector.memset(warm_sb[:], 1.0)
                warm_ps = pspool.tile([P, P], f32, tag="warm")
                for _ in range(n_warm):
                    nc.tensor.matmul(
                        out=warm_ps[:],
                        lhsT=warm_sb[:],
                        rhs=warm_sb[:],
                        start=True,
                        stop=True,
                    )

            first = True
            for _ in range(repeat):
                # chunked loads on the Sync HWDGE ring; consts slot in
                # right after the first chunk so compute starts ASAP
                engs = {"sync": nc.sync, "scalar": nc.scalar,
                        "gpsimd": nc.gpsimd}
                vmap = {}  # block idx -> (tile, local col offset)
                for li, (b0, nb) in enumerate(load_runs):
                    leng = engs[load_engs[li % len(load_engs)]]
                    vt = vpool.tile([D, max_load_nb * NB], f16, tag="v")
                    c0 = blocks[b0][0]
                    cw = sum(w for _, w in blocks[b0 : b0 + nb])
                    leng.dma_start(out=vt[:, :cw], in_=vT_in[:, c0 : c0 + cw])
                    for j in range(nb):
                        vmap[b0 + j] = (vt, blocks[b0 + j][0] - c0)
                    if first and li == 0:
                        nc.scalar.dma_start(out=wv_sb[:], in_=wvT_in[:])
                    if first and li == min(1, len(load_runs) - 1):
                        nc.scalar.dma_start(out=bT_sb[:], in_=bT_in[:])
                        first = False

                # PSUM->SBUF bias-add copies run over PAIRS of blocks (one
                # 2-bank PSUM tile per pair) to halve the per-op overhead,
                # alternating Vector / Scalar(ACT)
                pw = 2 if pair_copy else 1
                ci = 0
                for gi, (g0, gnb) in enumerate(store_runs):
                    h_sb = hpool.tile([D, max_store_nb * NB], f16, tag="h")
                    bi = g0
                    while bi < g0 + gnb:
                        pn = min(pw, g0 + gnb - bi)  # blocks in this copy
                        ps = pspool.tile([D, pw * NB], f32, tag="ps")
                        tot = 0
                        for j in range(pn):
                            col, w = blocks[bi + j]
                            vt, off = vmap[bi + j]
                            nc.tensor.matmul(
                                out=ps[:, j * NB : j * NB + w],
                                lhsT=wv_sb[:],
                                rhs=vt[:, off : off + w],
                                start=True,
                                stop=True,
                            )
                            tot = j * NB + w
                        hoff = (bi - g0) * NB
                        if ci % 2 == 0:
                            nc.vector.tensor_scalar_add(
                                h_sb[:, hoff : hoff + tot], ps[:, :tot],
                                bT_sb[:],
                            )
                        else:
                            # ACT: out = Identity(in * 1.0 + bias)
                            nc.scalar.add(
                                h_sb[:, hoff : hoff + tot], ps[:, :tot],
                                bT_sb[:],
                            )
                        ci += 1
                        bi += pn
                    c0 = blocks[g0][0]
                    cw = sum(w for _, w in blocks[g0 : g0 + gnb])
                    seng = engs[store_engs[gi % len(store_engs)]]
                    seng.dma_start(
                        out=hT_out[:, c0 : c0 + cw], in_=h_sb[:, :cw]
                    )

    nc.compile()
    return nc


def _build_module_masked(n_tiles, pad_idx):
    """Fallback SPMD program: h = (v @ wvT + b) * mask, all f32.

    n_tiles: 128-row node tiles per core (v/h are [n_tiles*128, 128]).
    pad_idx: padded per-core edge count for the mask histogram (multiple of
        128).
    """
    f32 = mybir.dt.float32
    NP = n_tiles * P

    nc = bacc.Bacc("TRN2", target_bir_lowering=False, debug=False)
    # node features arrive transposed ([D, nodes]) so tiles feed the PE's
    # lhsT port directly (contraction dim on partitions), full-line DMA
    vT_in = nc.dram_tensor("vT", [D, NP], f32, kind="ExternalInput")
    wvT_in = nc.dram_tensor("wvT", [D, D], f32, kind="ExternalInput")
    b_in = nc.dram_tensor("b", [1, D], f32, kind="ExternalInput")
    h_out = nc.dram_tensor("h", [NP, D], f32, kind="ExternalOutput")
    # num_idxs is a uint16 ISA field: split the histogram into chunks.
    n_chunks, per_chunk = _chunking(pad_idx)
    cols = per_chunk // 16
    idxs_in = nc.dram_tensor(
        "idxs", [P, n_chunks, cols], mybir.dt.int16, kind="ExternalInput"
    )
    # ExternalOutput: the runtime hands the kernel a pre-zeroed buffer,
    # which the scatter-add then accumulates into.
    table_out = nc.dram_tensor("tbl", [NP, TABLE_W], f32, kind="ExternalOutput")

    group = 4
    n_groups = -(-n_tiles // group)

    with tile.TileContext(nc) as tc:
        with (
            tc.tile_pool(name="const", bufs=1) as cpool,
            tc.tile_pool(name="vg", bufs=3) as vpool,
            tc.tile_pool(name="hg", bufs=3) as hpool,
            tc.tile_pool(name="psh", bufs=6, space="PSUM") as pspool_h,
        ):
            nc.gpsimd.load_library(mlp)
            idxs_sb = cpool.tile([P, n_chunks, cols], mybir.dt.int16)
            nc.sync.dma_start(out=idxs_sb[:], in_=idxs_in[:])
            ones_src = cpool.tile([P, per_chunk // P, 1], f32)
            nc.gpsimd.memset(ones_src[:], 1.0)
            # The SWDGE scatter-add ISA struct cannot carry sync waits;
            # absorb the idxs-DMA dependency on a cheap gpsimd op first.
            dep_sink = cpool.tile([P, 8], mybir.dt.int16)
            nc.gpsimd.tensor_copy(out=dep_sink[:], in_=idxs_sb[:, 0, :8])
            for ch in range(n_chunks):
                nc.gpsimd.dma_scatter_add(
                    table_out[:, 0:1],
                    ones_src[:],
                    idxs_sb[:, ch, :],
                    per_chunk,
                    per_chunk,
                    1,
                    elem_step=TABLE_W,
                )
            tblr_sb = cpool.tile([P, n_tiles * TABLE_W], f32)
            nc.sync.dma_start(
                out=tblr_sb[:].rearrange("p (t e) -> p t e", e=TABLE_W),
                in_=table_out[:].rearrange("(p t) e -> p t e", p=P),
            )
            mask_sb = cpool.tile([P, n_tiles], f32)
            counts_view = tblr_sb[:].rearrange(
                "p (t e) -> p t e", e=TABLE_W
            )[:, :, 0:1]
            nc.vector.tensor_scalar(
                out=mask_sb[:],
                in0=counts_view,
                scalar1=0.0,
                scalar2=None,
                op0=mybir.AluOpType.is_gt,
            )

            wvT_sb = cpool.tile([D, D], f32)
            nc.sync.dma_start(out=wvT_sb[:], in_=wvT_in[:])
            b_sb = cpool.tile([1, D], f32)
            nc.sync.dma_start(out=b_sb[:], in_=b_in[:])
            ones_row = cpool.tile([1, P], f32)
            nc.vector.memset(ones_row[:], 1.0)

            for g in range(n_groups):
                t0 = g * group
                gt = min(group, n_tiles - t0)
                v_sb = vpool.tile([P, group * D], f32, tag="vg")
                nc.sync.dma_start(
                    out=v_sb[:, : gt * D], in_=vT_in[:, t0 * D : (t0 + gt) * D]
                )
                h_sb = hpool.tile([P, group * D], f32, tag="hg")
                for i in range(gt):
                    t = t0 + i
                    h_ps = pspool_h.tile([P, P], f32, tag="hps")
                    nc.tensor.matmul(
                        out=h_ps[:],
                        lhsT=v_sb[:, i * P : (i + 1) * P],
                        rhs=wvT_sb[:],
                        start=True,
                        stop=False,
                    )
                    nc.tensor.matmul(
                        out=h_ps[:], lhsT=ones_row[:], rhs=b_sb[:],
                        start=False, stop=True,
                    )
                    nc.vector.tensor_scalar_mul(
                        h_sb[:, i * D : (i + 1) * D], h_ps[:],
                        mask_sb[:, t : t + 1],
                    )
                nc.sync.dma_start(
                    out=h_out[t0 * P : (t0 + gt) * P, :].rearrange(
                        "(g p) d -> p g d", p=P
                    ),
                    in_=h_sb[:, : gt * D].rearrange("p (g d) -> p g d", d=D),
                )

    nc.compile()
    return nc


def _get_module(key, builder, *args):
    if key not in _module_cache:
        _module_cache[key] = builder(*args)
    return _module_cache[key]


def kernel(V, E, edge_index, Wq_w, Wq_b, Wk_w, Wk_b, Wv_w, Wv_b, We_w, We_b,
           a_w, a_b, _trace=False):
    V = np.ascontiguousarray(np.asarray(V, dtype=np.float32))
    n_nodes, d = V.shape
    assert d == D and n_nodes % N_CORES == 0
    npc = n_nodes // N_CORES          # nodes per core
    n_tiles = -(-npc // P)            # 128-row tiles per core
    NP = n_tiles * P

    dest = np.asarray(edge_index)[1].astype(np.int64)
    counts = np.bincount(dest, minlength=n_nodes)
    covered = bool(counts.min() > 0)

    if covered:
        wvT = np.ascontiguousarray(np.asarray(Wv_w, dtype=np.float32).T
                                   ).astype(np.float16)
        bT = np.ascontiguousarray(
            np.asarray(Wv_b, dtype=np.float32)[:, None])
        in_maps = []
        for c in range(N_CORES):
            vpT = np.zeros((D, NP), dtype=np.float16)
            vpT[:, :npc] = V[c * npc : (c + 1) * npc].T
            in_maps.append({"vT": vpT, "wvT": wvT, "bT": bT})
        nc = _get_module(("fast", NP), _build_module_fast, NP)
        res = run_bass_kernel_spmd(nc, in_maps, core_ids=list(range(N_CORES)),
                                   trace=_trace)
        out = np.concatenate(
            [res.results[c]["hT"][:, :npc].T.astype(np.float32)
             for c in range(N_CORES)], axis=0)
        if _trace:
            return out, res
        return out

    # ---- fallback: some node has no incoming edge ----
    wvT = np.ascontiguousarray(np.asarray(Wv_w, dtype=np.float32).T)
    brow = np.ascontiguousarray(np.asarray(Wv_b, dtype=np.float32)[None, :])

    # dest-partition the edges; per-core local histogram indices,
    # permuted to the table layout row = (n%128)*n_tiles + n//128.
    core_of = dest // npc
    locs = []
    for c in range(N_CORES):
        n_loc = dest[core_of == c] - c * npc
        if len(n_loc) > 20 * MAX_IDXS_PER_SCATTER:
            # beyond the HW-validated per-core scatter envelope (extreme
            # dest skew): scatter only the distinct local nodes instead
            n_loc = np.unique(n_loc)
        locs.append(((n_loc % P) * n_tiles + n_loc // P).astype(np.int16))
    max_cnt = max(len(x) for x in locs)
    pad_idx = -(-max_cnt // P) * P
    n_chunks, per_chunk = _chunking(pad_idx)
    cols = per_chunk // 16

    in_maps = []
    for c in range(N_CORES):
        vpT = np.zeros((D, NP), dtype=np.float32)
        vpT[:, :npc] = V[c * npc : (c + 1) * npc].T
        m = {"vT": vpT, "wvT": wvT, "b": brow}
        # real indices first, then trailing -1 pads; chunked so pads are
        # trailing within each chunk (the SWDGE trims trailing negatives)
        flat = np.full(n_chunks * per_chunk, -1, dtype=np.int16)
        flat[: len(locs[c])] = locs[c]
        chunks = [
            np.tile(np.ascontiguousarray(ck.reshape(cols, 16).T), (N_CORES, 1))
            for ck in flat.reshape(n_chunks, per_chunk)
        ]
        m["idxs"] = np.ascontiguousarray(np.stack(chunks, axis=1))
        in_maps.append(m)

    nc = _get_module(("masked", n_tiles, pad_idx), _build_module_masked,
                     n_tiles, pad_idx)
    res = run_bass_kernel_spmd(nc, in_maps, core_ids=list(range(N_CORES)),
                               trace=_trace)
    out = np.concatenate([res.results[c]["h"][:npc] for c in range(N_CORES)],
                         axis=0)
    if _trace:
        return out, res
    return out


# revision 28
# speedup vs baseline: 1.0107x; 1.0107x over previous
"""GATv2 layer kernel for Trainium2 (8 NeuronCores, SPMD).

Math note: in the reference, the per-edge value vectors are gathered from the
*destination* node (Vv = V[dest] @ Wv^T + bv) and the scatter-softmax is also
grouped by destination. Within a destination segment Vv is constant, and the
softmax weights sum to 1, so

    H[n] = (V[n] @ Wv_w^T + Wv_b) * [n has >= 1 incoming edge]

exactly (up to f32 rounding of the softmax-weight sum, ~1e-7 relative).

Sharding: nodes are partitioned contiguously across the 8 cores; the small
[128,128] weight is replicated. Each core computes the Wv projection of its
node shard.

Fast path (every node has >= 1 incoming edge — true with overwhelming
probability at E/N = 12.5, and checked on host): the projection is computed
transposed, hT = Wv @ V_shard^T, with the weight as the PE's stationary
operand and 512-node column blocks of V^T streamed through as the moving
operand. I/O is fp16 (inputs quantized on host, output upcast on host),
halving HBM traffic; bias-add rides the PSUM->SBUF cast on the DVE as a
per-partition scalar. Well within the 2e-2 tolerance (~1e-3 max rel err).

Fallback (some node uncovered): the original f32 masked variant — node-tiled
matmuls plus a GPSIMD SWDGE scatter-add histogram over local edge
destinations to build the incoming-edge mask on-device.
"""

import numpy as np

import concourse.bacc as bacc
import concourse.bass as bass
import concourse.mybir as mybir
import concourse.tile as tile
from concourse.bass_utils import run_bass_kernel_spmd
from concourse.library_config import mlp

N_CORES = 8
P = 128
D = 128
NB = 512  # node block: one PSUM bank of f32, max f32 moving free-dim
TABLE_W = 64  # f32 words per histogram-table row -> 256B stride (SWDGE req.)

_module_cache = {}

# Cap indices per SWDGE scatter-add: the Q7 expands indices to int32 in
# local scratch (4096 validated on HW; 8192 crashes the exec unit).
MAX_IDXS_PER_SCATTER = 4096


def _chunking(pad_idx):
    n_chunks = -(-pad_idx // MAX_IDXS_PER_SCATTER)
    per_chunk = -(-pad_idx // (n_chunks * P)) * P
    return n_chunks, per_chunk


def _even_split(n, k):
    """Split n items into k contiguous runs, sizes as even as possible."""
    base, rem = divmod(n, k)
    runs = []
    start = 0
    for i in range(k):
        sz = base + (1 if i < rem else 0)
        runs.append((start, sz))
        start += sz
    return [r for r in runs if r[1] > 0]


def _build_module_fast(NP):
    """Maskless SPMD program: hT = wvT.T @ vT + b, fp16 in/out.

    NP: padded per-core node count (multiple of 128). vT/hT are [D, NP].
    Loads ride the Sync HWDGE ring, stores the Scalar (ACT) ring, so input
    and output streams issue in parallel; the PSUM->SBUF bias-add copies
    alternate between Vector and GpSimd.
    """
    import os

    f32 = mybir.dt.float32
    f16 = mybir.dt.float16

    psbufs = int(os.environ.get("K_PSBUFS", "6"))
    repeat = int(os.environ.get("K_REPEAT", "1"))  # timing experiments only
    n_warm = int(os.environ.get("K_NWARM", "0"))
    # per-load-chunk / per-store-group issuing engines. L0 gets the Sync
    # HWDGE ring to itself so its completion isn't delayed by packet
    # round-robin with later chunks; remaining loads split between the ACT
    # HWDGE ring and GPSIMD's SWDGE queues; stores reuse the Sync ring,
    # which is idle again by the time the first store issues.
    # all loads on ONE HWDGE ring: blocks arrive strictly in order (the PE
    # consumes them in order, so out-of-order arrival head-of-line blocks)
    # and the ring drains at full SDMA attention. Stores stream concurrently
    # on GPSIMD's SWDGE queue row; the last store takes the ACT HWDGE ring
    # for its faster completion receipt.
    load_engs = os.environ.get("K_LENG", "sync").split(",")
    store_engs = os.environ.get(
        "K_SENG", "gpsimd,gpsimd,gpsimd,gpsimd,gpsimd,scalar").split(",")
    pair_copy = os.environ.get("K_PAIRCOPY", "0") == "1"

    # node blocks: one f32 PSUM bank each (= max f32 moving free-dim)
    blocks = []
    col = 0
    while col < NP:
        w = min(NB, NP - col)
        blocks.append((col, w))
        col += w
    n_blocks = len(blocks)

    def _runs(spec_env, default):
        spec = [int(x) for x in os.environ.get(spec_env, default).split(",")]
        runs = []
        b = 0
        for sz in spec:
            if b >= n_blocks:
                break
            sz = min(sz, n_blocks - b)
            runs.append((b, sz))
            b += sz
        while b < n_blocks:  # spill: repeat last size
            sz = min(spec[-1], n_blocks - b)
            runs.append((b, sz))
            b += sz
        return runs

    # first load chunks small -> compute starts early and never stalls on
    # a big chunk; first store small -> write stream overlaps read stream;
    # last store small -> short drain tail
    load_runs = _runs("K_LOADS", "1,2,2,4,3,1")
    store_runs = _runs("K_STORES", "2,2,3,3,2,1")
    max_load_nb = max(nb for _, nb in load_runs)
    max_store_nb = max(nb for _, nb in store_runs)

    nc = bacc.Bacc("TRN2", target_bir_lowering=False, debug=False)
    vT_in = nc.dram_tensor("vT", [D, NP], f16, kind="ExternalInput")
    wvT_in = nc.dram_tensor("wvT", [D, D], f16, kind="ExternalInput")
    bT_in = nc.dram_tensor("bT", [D, 1], f32, kind="ExternalInput")
    hT_out = nc.dram_tensor("hT", [D, NP], f16, kind="ExternalOutput")

    with tile.TileContext(nc) as tc:
        with (
            tc.tile_pool(name="const", bufs=1) as cpool,
            tc.tile_pool(name="vg", bufs=len(load_runs) + 1) as vpool,
            tc.tile_pool(name="hg", bufs=len(store_runs) + 1) as hpool,
            tc.tile_pool(name="psh", bufs=psbufs, space="PSUM") as pspool,
        ):
            wv_sb = cpool.tile([D, D], f16)
            bT_sb = cpool.tile([D, 1], f32)

            # HAM warm-up: ~4us of back-to-back dummy matmuls bridges the
            # PE from the engine-start barrier to the first real matmul so
            # the 2.4 GHz clock-gate is open (and stays open; the real
            # matmul cadence never idles past the ~3.4us MID window).
            if n_warm:
                warm_sb = cpool.tile([P, P], f16)
                nc.vector.memset(warm_sb[:], 1.0)
                warm_ps = pspool.tile([P, P], f32, tag="warm")
                for _ in range(n_warm):
                    nc.tensor.matmul(
                        out=warm_ps[:],
                        lhsT=warm_sb[:],
                        rhs=warm_sb[:],
                        start=True,
                        stop=True,
                    )

            first = True
            for _ in range(repeat):
                # chunked loads on the Sync HWDGE ring; consts slot in
                # right after the first chunk so compute starts ASAP
                engs = {"sync": nc.sync, "scalar": nc.scalar,
                        "gpsimd": nc.gpsimd}
                vmap = {}  # block idx -> (tile, local col offset)
                for li, (b0, nb) in enumerate(load_runs):
                    leng = engs[load_engs[li % len(load_engs)]]
                    vt = vpool.tile([D, max_load_nb * NB], f16, tag="v")
                    c0 = blocks[b0][0]
                    cw = sum(w for _, w in blocks[b0 : b0 + nb])
                    leng.dma_start(out=vt[:, :cw], in_=vT_in[:, c0 : c0 + cw])
                    for j in range(nb):
                        vmap[b0 + j] = (vt, blocks[b0 + j][0] - c0)
                    if first and li == 0:
                        nc.scalar.dma_start(out=wv_sb[:], in_=wvT_in[:])
                    if first and li == min(1, len(load_runs) - 1):
                        nc.scalar.dma_start(out=bT_sb[:], in_=bT_in[:])
                        first = False

                # PSUM->SBUF bias-add copies run over PAIRS of blocks (one
                # 2-bank PSUM tile per pair) to halve the per-op overhead,
                # alternating Vector / Scalar(ACT)
                pw = 2 if pair_copy else 1
                ci = 0
                for gi, (g0, gnb) in enumerate(store_runs):
                    h_sb = hpool.tile([D, max_store_nb * NB], f16, tag="h")
                    bi = g0
                    while bi < g0 + gnb:
                        pn = min(pw, g0 + gnb - bi)  # blocks in this copy
                        ps = pspool.tile([D, pw * NB], f32, tag="ps")
                        tot = 0
                        for j in range(pn):
                            col, w = blocks[bi + j]
                            vt, off = vmap[bi + j]
                            nc.tensor.matmul(
                                out=ps[:, j * NB : j * NB + w],
                                lhsT=wv_sb[:],
                                rhs=vt[:, off : off + w],
                                start=True,
                                stop=True,
                            )
                            tot = j * NB + w
                        hoff = (bi - g0) * NB
                        if ci % 2 == 0:
                            nc.vector.tensor_scalar_add(
                                h_sb[:, hoff : hoff + tot], ps[:, :tot],
                                bT_sb[:],
                            )
                        else:
                            # ACT: out = Identity(in * 1.0 + bias)
                            nc.scalar.add(
                                h_sb[:, hoff : hoff + tot], ps[:, :tot],
                                bT_sb[:],
                            )
                        ci += 1
                        bi += pn
                    c0 = blocks[g0][0]
                    cw = sum(w for _, w in blocks[g0 : g0 + gnb])
                    seng = engs[store_engs[gi % len(store_engs)]]
                    seng.dma_start(
                        out=hT_out[:, c0 : c0 + cw], in_=h_sb[:, :cw]
                    )

    nc.compile()
    return nc


def _build_module_masked(n_tiles, pad_idx):
    """Fallback SPMD program: h = (v @ wvT + b) * mask, all f32.

    n_tiles: 128-row node tiles per core (v/h are [n_tiles*128, 128]).
    pad_idx: padded per-core edge count for the mask histogram (multiple of
        128).
    """
    f32 = mybir.dt.float32
    NP = n_tiles * P

    nc = bacc.Bacc("TRN2", target_bir_lowering=False, debug=False)
    # node features arrive transposed ([D, nodes]) so tiles feed the PE's
    # lhsT port directly (contraction dim on partitions), full-line DMA
    vT_in = nc.dram_tensor("vT", [D, NP], f32, kind="ExternalInput")
    wvT_in = nc.dram_tensor("wvT", [D, D], f32, kind="ExternalInput")
    b_in = nc.dram_tensor("b", [1, D], f32, kind="ExternalInput")
    h_out = nc.dram_tensor("h", [NP, D], f32, kind="ExternalOutput")
    # num_idxs is a uint16 ISA field: split the histogram into chunks.
    n_chunks, per_chunk = _chunking(pad_idx)
    cols = per_chunk // 16
    idxs_in = nc.dram_tensor(
        "idxs", [P, n_chunks, cols], mybir.dt.int16, kind="ExternalInput"
    )
    # ExternalOutput: the runtime hands the kernel a pre-zeroed buffer,
    # which the scatter-add then accumulates into.
    table_out = nc.dram_tensor("tbl", [NP, TABLE_W], f32, kind="ExternalOutput")

    group = 4
    n_groups = -(-n_tiles // group)

    with tile.TileContext(nc) as tc:
        with (
            tc.tile_pool(name="const", bufs=1) as cpool,
            tc.tile_pool(name="vg", bufs=3) as vpool,
            tc.tile_pool(name="hg", bufs=3) as hpool,
            tc.tile_pool(name="psh", bufs=6, space="PSUM") as pspool_h,
        ):
            nc.gpsimd.load_library(mlp)
            idxs_sb = cpool.tile([P, n_chunks, cols], mybir.dt.int16)
            nc.sync.dma_start(out=idxs_sb[:], in_=idxs_in[:])
            ones_src = cpool.tile([P, per_chunk // P, 1], f32)
            nc.gpsimd.memset(ones_src[:], 1.0)
            # The SWDGE scatter-add ISA struct cannot carry sync waits;
            # absorb the idxs-DMA dependency on a cheap gpsimd op first.
            dep_sink = cpool.tile([P, 8], mybir.dt.int16)
            nc.gpsimd.tensor_copy(out=dep_sink[:], in_=idxs_sb[:, 0, :8])
            for ch in range(n_chunks):
                nc.gpsimd.dma_scatter_add(
                    table_out[:, 0:1],
                    ones_src[:],
                    idxs_sb[:, ch, :],
                    per_chunk,
                    per_chunk,
                    1,
                    elem_step=TABLE_W,
                )
            tblr_sb = cpool.tile([P, n_tiles * TABLE_W], f32)
            nc.sync.dma_start(
                out=tblr_sb[:].rearrange("p (t e) -> p t e", e=TABLE_W),
                in_=table_out[:].rearrange("(p t) e -> p t e", p=P),
            )
            mask_sb = cpool.tile([P, n_tiles], f32)
            counts_view = tblr_sb[:].rearrange(
                "p (t e) -> p t e", e=TABLE_W
            )[:, :, 0:1]
            nc.vector.tensor_scalar(
                out=mask_sb[:],
                in0=counts_view,
                scalar1=0.0,
                scalar2=None,
                op0=mybir.AluOpType.is_gt,
            )

            wvT_sb = cpool.tile([D, D], f32)
            nc.sync.dma_start(out=wvT_sb[:], in_=wvT_in[:])
            b_sb = cpool.tile([1, D], f32)
            nc.sync.dma_start(out=b_sb[:], in_=b_in[:])
            ones_row = cpool.tile([1, P], f32)
            nc.vector.memset(ones_row[:], 1.0)

            for g in range(n_groups):
                t0 = g * group
                gt = min(group, n_tiles - t0)
                v_sb = vpool.tile([P, group * D], f32, tag="vg")
                nc.sync.dma_start(
                    out=v_sb[:, : gt * D], in_=vT_in[:, t0 * D : (t0 + gt) * D]
                )
                h_sb = hpool.tile([P, group * D], f32, tag="hg")
                for i in range(gt):
                    t = t0 + i
                    h_ps = pspool_h.tile([P, P], f32, tag="hps")
                    nc.tensor.matmul(
                        out=h_ps[:],
                        lhsT=v_sb[:, i * P : (i + 1) * P],
                        rhs=wvT_sb[:],
                        start=True,
                        stop=False,
                    )
                    nc.tensor.matmul(
                        out=h_ps[:], lhsT=ones_row[:], rhs=b_sb[:],
                        start=False, stop=True,
                    )
                    nc.vector.tensor_scalar_mul(
                        h_sb[:, i * D : (i + 1) * D], h_ps[:],
                        mask_sb[:, t : t + 1],
                    )
                nc.sync.dma_start(
                    out=h_out[t0 * P : (t0 + gt) * P, :].rearrange(
                        "(g p) d -> p g d", p=P
                    ),
                    in_=h_sb[:, : gt * D].rearrange("p (g d) -> p g d", d=D),
                )

    nc.compile()
    return nc


def _get_module(key, builder, *args):
    if key not in _module_cache:
        _module_cache[key] = builder(*args)
    return _module_cache[key]


def kernel(V, E, edge_index, Wq_w, Wq_b, Wk_w, Wk_b, Wv_w, Wv_b, We_w, We_b,
           a_w, a_b, _trace=False):
    V = np.ascontiguousarray(np.asarray(V, dtype=np.float32))
    n_nodes, d = V.shape
    assert d == D and n_nodes % N_CORES == 0
    npc = n_nodes // N_CORES          # nodes per core
    n_tiles = -(-npc // P)            # 128-row tiles per core
    NP = n_tiles * P

    dest = np.asarray(edge_index)[1].astype(np.int64)
    counts = np.bincount(dest, minlength=n_nodes)
    covered = bool(counts.min() > 0)

    if covered:
        wvT = np.ascontiguousarray(np.asarray(Wv_w, dtype=np.float32).T
                                   ).astype(np.float16)
        bT = np.ascontiguousarray(
            np.asarray(Wv_b, dtype=np.float32)[:, None])
        in_maps = []
        for c in range(N_CORES):
            vpT = np.zeros((D, NP), dtype=np.float16)
            vpT[:, :npc] = V[c * npc : (c + 1) * npc].T
            in_maps.append({"vT": vpT, "wvT": wvT, "bT": bT})
        nc = _get_module(("fast", NP), _build_module_fast, NP)
        res = run_bass_kernel_spmd(nc, in_maps, core_ids=list(range(N_CORES)),
                                   trace=_trace)
        out = np.concatenate(
            [res.results[c]["hT"][:, :npc].T.astype(np.float32)
             for c in range(N_CORES)], axis=0)
        if _trace:
            return out, res
        return out

    # ---- fallback: some node has no incoming edge ----
    wvT = np.ascontiguousarray(np.asarray(Wv_w, dtype=np.float32).T)
    brow = np.ascontiguousarray(np.asarray(Wv_b, dtype=np.float32)[None, :])

    # dest-partition the edges; per-core local histogram indices,
    # permuted to the table layout row = (n%128)*n_tiles + n//128.
    core_of = dest // npc
    locs = []
    for c in range(N_CORES):
        n_loc = dest[core_of == c] - c * npc
        if len(n_loc) > 20 * MAX_IDXS_PER_SCATTER:
            # beyond the HW-validated per-core scatter envelope (extreme
            # dest skew): scatter only the distinct local nodes instead
            n_loc = np.unique(n_loc)
        locs.append(((n_loc % P) * n_tiles + n_loc // P).astype(np.int16))
    max_cnt = max(len(x) for x in locs)
    pad_idx = -(-max_cnt // P) * P
    n_chunks, per_chunk = _chunking(pad_idx)
    cols = per_chunk // 16

    in_maps = []
    for c in range(N_CORES):
        vpT = np.zeros((D, NP), dtype=np.float32)
        vpT[:, :npc] = V[c * npc : (c + 1) * npc].T
        m = {"vT": vpT, "wvT": wvT, "b": brow}
        # real indices first, then trailing -1 pads; chunked so pads are
        # trailing within each chunk (the SWDGE trims trailing negatives)
        flat = np.full(n_chunks * per_chunk, -1, dtype=np.int16)
        flat[: len(locs[c])] = locs[c]
        chunks = [
            np.tile(np.ascontiguousarray(ck.reshape(cols, 16).T), (N_CORES, 1))
            for ck in flat.reshape(n_chunks, per_chunk)
        ]
        m["idxs"] = np.ascontiguousarray(np.stack(chunks, axis=1))
        in_maps.append(m)

    nc = _get_module(("masked", n_tiles, pad_idx), _build_module_masked,
                     n_tiles, pad_idx)
    res = run_bass_kernel_spmd(nc, in_maps, core_ids=list(range(N_CORES)),
                               trace=_trace)
    out = np.concatenate([res.results[c]["h"][:npc] for c in range(N_CORES)],
                         axis=0)
    if _trace:
        return out, res
    return out


# revision 33
# speedup vs baseline: 1.0847x; 1.0732x over previous
"""GATv2 layer kernel for Trainium2 (8 NeuronCores, SPMD).

Math note: in the reference, the per-edge value vectors are gathered from the
*destination* node (Vv = V[dest] @ Wv^T + bv) and the scatter-softmax is also
grouped by destination. Within a destination segment Vv is constant, and the
softmax weights sum to 1, so

    H[n] = (V[n] @ Wv_w^T + Wv_b) * [n has >= 1 incoming edge]

exactly (up to f32 rounding of the softmax-weight sum, ~1e-7 relative).

Sharding: nodes are partitioned contiguously across the 8 cores; the small
[128,128] weight is replicated. Each core computes the Wv projection of its
node shard.

Fast path (every node has >= 1 incoming edge — true with overwhelming
probability at E/N = 12.5, and checked on host): the projection is computed
transposed, hT = Wv @ V_shard^T, with the weight as the PE's stationary
operand and 512-node column blocks of V^T streamed through as the moving
operand. I/O is fp16 (inputs quantized on host, output upcast on host),
halving HBM traffic; bias-add rides the PSUM->SBUF cast on the DVE as a
per-partition scalar. Well within the 2e-2 tolerance (~1e-3 max rel err).

Fallback (some node uncovered): the original f32 masked variant — node-tiled
matmuls plus a GPSIMD SWDGE scatter-add histogram over local edge
destinations to build the incoming-edge mask on-device.
"""

import numpy as np

import concourse.bacc as bacc
import concourse.bass as bass
import concourse.mybir as mybir
import concourse.tile as tile
from concourse.bass_utils import run_bass_kernel_spmd
from concourse.library_config import mlp

N_CORES = 8
P = 128
D = 128
NB = 512  # node block: one PSUM bank of f32, max f32 moving free-dim
TABLE_W = 64  # f32 words per histogram-table row -> 256B stride (SWDGE req.)

_module_cache = {}

# Cap indices per SWDGE scatter-add: the Q7 expands indices to int32 in
# local scratch (4096 validated on HW; 8192 crashes the exec unit).
MAX_IDXS_PER_SCATTER = 4096


def _chunking(pad_idx):
    n_chunks = -(-pad_idx // MAX_IDXS_PER_SCATTER)
    per_chunk = -(-pad_idx // (n_chunks * P)) * P
    return n_chunks, per_chunk


def _even_split(n, k):
    """Split n items into k contiguous runs, sizes as even as possible."""
    base, rem = divmod(n, k)
    runs = []
    start = 0
    for i in range(k):
        sz = base + (1 if i < rem else 0)
        runs.append((start, sz))
        start += sz
    return [r for r in runs if r[1] > 0]


def _build_module_fast(NP):
    """Maskless SPMD program: hT = wvT.T @ vT + b, fp16 in/out.

    NP: padded per-core node count (multiple of 128). vT/hT are [D, NP].
    Loads ride the Sync HWDGE ring, stores the Scalar (ACT) ring, so input
    and output streams issue in parallel; the PSUM->SBUF bias-add copies
    alternate between Vector and GpSimd.
    """
    import os

    f32 = mybir.dt.float32
    f16 = mybir.dt.float16

    psbufs = int(os.environ.get("K_PSBUFS", "6"))
    repeat = int(os.environ.get("K_REPEAT", "1"))  # timing experiments only
    n_warm = int(os.environ.get("K_NWARM", "48"))
    # per-load-chunk / per-store-group issuing engines. L0 gets the Sync
    # HWDGE ring to itself so its completion isn't delayed by packet
    # round-robin with later chunks; remaining loads split between the ACT
    # HWDGE ring and GPSIMD's SWDGE queues; stores reuse the Sync ring,
    # which is idle again by the time the first store issues.
    # Single-ring FIFO schedule: ALL loads then the bulk stores share the
    # Sync HWDGE ring. The SDMA engine pool (~350-420 GB/s aggregate) is the
    # real bottleneck, so "concurrent" read/write streams only steal read
    # bandwidth from the cascade; FIFO gives reads the full pool up front
    # (blocks arrive in order -- no head-of-line blocking at the PE) and
    # writes the full pool behind them. Only the tiny final store jumps to
    # the otherwise-idle ACT ring so it doesn't queue behind the bulk
    # stores.
    load_engs = os.environ.get("K_LENG", "sync").split(",")
    store_engs = os.environ.get(
        "K_SENG", "sync,sync,sync,scalar").split(",")
    pair_copy = os.environ.get("K_PAIRCOPY", "0") == "1"

    # node blocks: one f32 PSUM bank each (= max f32 moving free-dim)
    blocks = []
    col = 0
    while col < NP:
        w = min(NB, NP - col)
        blocks.append((col, w))
        col += w
    n_blocks = len(blocks)

    def _runs(spec_env, default):
        spec = [int(x) for x in os.environ.get(spec_env, default).split(",")]
        runs = []
        b = 0
        for sz in spec:
            if b >= n_blocks:
                break
            sz = min(sz, n_blocks - b)
            runs.append((b, sz))
            b += sz
        while b < n_blocks:  # spill: repeat last size
            sz = min(spec[-1], n_blocks - b)
            runs.append((b, sz))
            b += sz
        return runs

    # first load chunks small -> compute starts early and never stalls on
    # a big chunk; first store small -> write stream overlaps read stream;
    # last store small -> short drain tail
    load_runs = _runs("K_LOADS", "2,3,4,3,1")
    store_runs = _runs("K_STORES", "4,4,4,1")
    max_load_nb = max(nb for _, nb in load_runs)
    max_store_nb = max(nb for _, nb in store_runs)

    nc = bacc.Bacc("TRN2", target_bir_lowering=False, debug=False)
    vT_in = nc.dram_tensor("vT", [D, NP], f16, kind="ExternalInput")
    wvT_in = nc.dram_tensor("wvT", [D, D], f16, kind="ExternalInput")
    bT_in = nc.dram_tensor("bT", [D, 1], f32, kind="ExternalInput")
    hT_out = nc.dram_tensor("hT", [D, NP], f16, kind="ExternalOutput")

    with tile.TileContext(nc) as tc:
        with (
            tc.tile_pool(name="const", bufs=1) as cpool,
            tc.tile_pool(name="vg", bufs=len(load_runs) + 1) as vpool,
            tc.tile_pool(name="hg", bufs=len(store_runs) + 1) as hpool,
            tc.tile_pool(name="psh", bufs=psbufs, space="PSUM") as pspool,
            tc.tile_pool(name="pswarm", bufs=1, space="PSUM") as warmpool,
        ):
            wv_sb = cpool.tile([D, D], f16)
            bT_sb = cpool.tile([D, 1], f32)

            # HAM warm-up: ~4us of back-to-back dummy matmuls bridges the
            # PE from the engine-start barrier to the first real matmul so
            # the 2.4 GHz clock-gate is open (and stays open; the real
            # matmul cadence never idles past the ~3.4us MID window).
            if n_warm:
                warm_sb = cpool.tile([P, P], f16)
                nc.vector.memset(warm_sb[:], 1.0)
                warm_ps = warmpool.tile([P, P], f32, tag="warm")
                for _ in range(n_warm):
                    nc.tensor.matmul(
                        out=warm_ps[:],
                        lhsT=warm_sb[:],
                        rhs=warm_sb[:],
                        start=True,
                        stop=True,
                    )

            first = True
            for _ in range(repeat):
                # chunked loads on the Sync HWDGE ring; consts slot in
                # right after the first chunk so compute starts ASAP
                engs = {"sync": nc.sync, "scalar": nc.scalar,
                        "gpsimd": nc.gpsimd}
                vmap = {}  # block idx -> (tile, local col offset)
                for li, (b0, nb) in enumerate(load_runs):
                    leng = engs[load_engs[li % len(load_engs)]]
                    vt = vpool.tile([D, max_load_nb * NB], f16, tag="v")
                    c0 = blocks[b0][0]
                    cw = sum(w for _, w in blocks[b0 : b0 + nb])
                    leng.dma_start(out=vt[:, :cw], in_=vT_in[:, c0 : c0 + cw])
                    for j in range(nb):
                        vmap[b0 + j] = (vt, blocks[b0 + j][0] - c0)
                    if first and li == 0:
                        nc.scalar.dma_start(out=wv_sb[:], in_=wvT_in[:])
                    if first and li == min(1, len(load_runs) - 1):
                        nc.scalar.dma_start(out=bT_sb[:], in_=bT_in[:])
                        first = False

                # PSUM->SBUF bias-add copies run over PAIRS of blocks (one
                # 2-bank PSUM tile per pair) to halve the per-op overhead,
                # alternating Vector / Scalar(ACT)
                pw = 2 if pair_copy else 1
                ci = 0
                for gi, (g0, gnb) in enumerate(store_runs):
                    h_sb = hpool.tile([D, max_store_nb * NB], f16, tag="h")
                    bi = g0
                    while bi < g0 + gnb:
                        pn = min(pw, g0 + gnb - bi)  # blocks in this copy
                        ps = pspool.tile([D, pw * NB], f32, tag="ps")
                        tot = 0
                        for j in range(pn):
                            col, w = blocks[bi + j]
                            vt, off = vmap[bi + j]
                            nc.tensor.matmul(
                                out=ps[:, j * NB : j * NB + w],
                                lhsT=wv_sb[:],
                                rhs=vt[:, off : off + w],
                                start=True,
                                stop=True,
                            )
                            tot = j * NB + w
                        hoff = (bi - g0) * NB
                        if ci % 2 == 0:
                            nc.vector.tensor_scalar_add(
                                h_sb[:, hoff : hoff + tot], ps[:, :tot],
                                bT_sb[:],
                            )
                        else:
                            # ACT: out = Identity(in * 1.0 + bias)
                            nc.scalar.add(
                                h_sb[:, hoff : hoff + tot], ps[:, :tot],
                                bT_sb[:],
                            )
                        ci += 1
                        bi += pn
                    c0 = blocks[g0][0]
                    cw = sum(w for _, w in blocks[g0 : g0 + gnb])
                    seng = engs[store_engs[gi % len(store_engs)]]
                    seng.dma_start(
                        out=hT_out[:, c0 : c0 + cw], in_=h_sb[:, :cw]
                    )

    nc.compile()
    return nc


def _build_module_masked(n_tiles, pad_idx):
    """Fallback SPMD program: h = (v @ wvT + b) * mask, all f32.

    n_tiles: 128-row node tiles per core (v/h are [n_tiles*128, 128]).
    pad_idx: padded per-core edge count for the mask histogram (multiple of
        128).
    """
    f32 = mybir.dt.float32
    NP = n_tiles * P

    nc = bacc.Bacc("TRN2", target_bir_lowering=False, debug=False)
    # node features arrive transposed ([D, nodes]) so tiles feed the PE's
    # lhsT port directly (contraction dim on partitions), full-line DMA
    vT_in = nc.dram_tensor("vT", [D, NP], f32, kind="ExternalInput")
    wvT_in = nc.dram_tensor("wvT", [D, D], f32, kind="ExternalInput")
    b_in = nc.dram_tensor("b", [1, D], f32, kind="ExternalInput")
    h_out = nc.dram_tensor("h", [NP, D], f32, kind="ExternalOutput")
    # num_idxs is a uint16 ISA field: split the histogram into chunks.
    n_chunks, per_chunk = _chunking(pad_idx)
    cols = per_chunk // 16
    idxs_in = nc.dram_tensor(
        "idxs", [P, n_chunks, cols], mybir.dt.int16, kind="ExternalInput"
    )
    # ExternalOutput: the runtime hands the kernel a pre-zeroed buffer,
    # which the scatter-add then accumulates into.
    table_out = nc.dram_tensor("tbl", [NP, TABLE_W], f32, kind="ExternalOutput")

    group = 4
    n_groups = -(-n_tiles // group)

    with tile.TileContext(nc) as tc:
        with (
            tc.tile_pool(name="const", bufs=1) as cpool,
            tc.tile_pool(name="vg", bufs=3) as vpool,
            tc.tile_pool(name="hg", bufs=3) as hpool,
            tc.tile_pool(name="psh", bufs=6, space="PSUM") as pspool_h,
        ):
            nc.gpsimd.load_library(mlp)
            idxs_sb = cpool.tile([P, n_chunks, cols], mybir.dt.int16)
            nc.sync.dma_start(out=idxs_sb[:], in_=idxs_in[:])
            ones_src = cpool.tile([P, per_chunk // P, 1], f32)
            nc.gpsimd.memset(ones_src[:], 1.0)
            # The SWDGE scatter-add ISA struct cannot carry sync waits;
            # absorb the idxs-DMA dependency on a cheap gpsimd op first.
            dep_sink = cpool.tile([P, 8], mybir.dt.int16)
            nc.gpsimd.tensor_copy(out=dep_sink[:], in_=idxs_sb[:, 0, :8])
            for ch in range(n_chunks):
                nc.gpsimd.dma_scatter_add(
                    table_out[:, 0:1],
                    ones_src[:],
                    idxs_sb[:, ch, :],
                    per_chunk,
                    per_chunk,
                    1,
                    elem_step=TABLE_W,
                )
            tblr_sb = cpool.tile([P, n_tiles * TABLE_W], f32)
            nc.sync.dma_start(
                out=tblr_sb[:].rearrange("p (t e) -> p t e", e=TABLE_W),
                in_=table_out[:].rearrange("(p t) e -> p t e", p=P),
            )
            mask_sb = cpool.tile([P, n_tiles], f32)
            counts_view = tblr_sb[:].rearrange(
                "p (t e) -> p t e", e=TABLE_W
            )[:, :, 0:1]
            nc.vector.tensor_scalar(
                out=mask_sb[:],
                in0=counts_view,
                scalar1=0.0,
                scalar2=None,
                op0=mybir.AluOpType.is_gt,
            )

            wvT_sb = cpool.tile([D, D], f32)
            nc.sync.dma_start(out=wvT_sb[:], in_=wvT_in[:])
            b_sb = cpool.tile([1, D], f32)
            nc.sync.dma_start(out=b_sb[:], in_=b_in[:])
            ones_row = cpool.tile([1, P], f32)
            nc.vector.memset(ones_row[:], 1.0)

            for g in range(n_groups):
                t0 = g * group
                gt = min(group, n_tiles - t0)
                v_sb = vpool.tile([P, group * D], f32, tag="vg")
                nc.sync.dma_start(
                    out=v_sb[:, : gt * D], in_=vT_in[:, t0 * D : (t0 + gt) * D]
                )
                h_sb = hpool.tile([P, group * D], f32, tag="hg")
                for i in range(gt):
                    t = t0 + i
                    h_ps = pspool_h.tile([P, P], f32, tag="hps")
                    nc.tensor.matmul(
                        out=h_ps[:],
                        lhsT=v_sb[:, i * P : (i + 1) * P],
                        rhs=wvT_sb[:],
                        start=True,
                        stop=False,
                    )
                    nc.tensor.matmul(
                        out=h_ps[:], lhsT=ones_row[:], rhs=b_sb[:],
                        start=False, stop=True,
                    )
                    nc.vector.tensor_scalar_mul(
                        h_sb[:, i * D : (i + 1) * D], h_ps[:],
                        mask_sb[:, t : t + 1],
                    )
                nc.sync.dma_start(
                    out=h_out[t0 * P : (t0 + gt) * P, :].rearrange(
                        "(g p) d -> p g d", p=P
                    ),
                    in_=h_sb[:, : gt * D].rearrange("p (g d) -> p g d", d=D),
                )

    nc.compile()
    return nc


def _get_module(key, builder, *args):
    if key not in _module_cache:
        _module_cache[key] = builder(*args)
    return _module_cache[key]


def kernel(V, E, edge_index, Wq_w, Wq_b, Wk_w, Wk_b, Wv_w, Wv_b, We_w, We_b,
           a_w, a_b, _trace=False):
    V = np.ascontiguousarray(np.asarray(V, dtype=np.float32))
    n_nodes, d = V.shape
    assert d == D and n_nodes % N_CORES == 0
    npc = n_nodes // N_CORES          # nodes per core
    n_tiles = -(-npc // P)            # 128-row tiles per core
    NP = n_tiles * P

    dest = np.asarray(edge_index)[1].astype(np.int64)
    counts = np.bincount(dest, minlength=n_nodes)
    covered = bool(counts.min() > 0)

    if covered:
        wvT = np.ascontiguousarray(np.asarray(Wv_w, dtype=np.float32).T
                                   ).astype(np.float16)
        bT = np.ascontiguousarray(
            np.asarray(Wv_b, dtype=np.float32)[:, None])
        in_maps = []
        for c in range(N_CORES):
            vpT = np.zeros((D, NP), dtype=np.float16)
            vpT[:, :npc] = V[c * npc : (c + 1) * npc].T
            in_maps.append({"vT": vpT, "wvT": wvT, "bT": bT})
        nc = _get_module(("fast", NP), _build_module_fast, NP)
        res = run_bass_kernel_spmd(nc, in_maps, core_ids=list(range(N_CORES)),
                                   trace=_trace)
        out = np.concatenate(
            [res.results[c]["hT"][:, :npc].T.astype(np.float32)
             for c in range(N_CORES)], axis=0)
        if _trace:
            return out, res
        return out

    # ---- fallback: some node has no incoming edge ----
    wvT = np.ascontiguousarray(np.asarray(Wv_w, dtype=np.float32).T)
    brow = np.ascontiguousarray(np.asarray(Wv_b, dtype=np.float32)[None, :])

    # dest-partition the edges; per-core local histogram indices,
    # permuted to the table layout row = (n%128)*n_tiles + n//128.
    core_of = dest // npc
    locs = []
    for c in range(N_CORES):
        n_loc = dest[core_of == c] - c * npc
        if len(n_loc) > 20 * MAX_IDXS_PER_SCATTER:
            # beyond the HW-validated per-core scatter envelope (extreme
            # dest skew): scatter only the distinct local nodes instead
            n_loc = np.unique(n_loc)
        locs.append(((n_loc % P) * n_tiles + n_loc // P).astype(np.int16))
    max_cnt = max(len(x) for x in locs)
    pad_idx = -(-max_cnt // P) * P
    n_chunks, per_chunk = _chunking(pad_idx)
    cols = per_chunk // 16

    in_maps = []
    for c in range(N_CORES):
        vpT = np.zeros((D, NP), dtype=np.float32)
        vpT[:, :npc] = V[c * npc : (c + 1) * npc].T
        m = {"vT": vpT, "wvT": wvT, "b": brow}
        # real indices first, then trailing -1 pads; chunked so pads are
        # trailing within each chunk (the SWDGE trims trailing negatives)
        flat = np.full(n_chunks * per_chunk, -1, dtype=np.int16)
        flat[: len(locs[c])] = locs[c]
        chunks = [
            np.tile(np.ascontiguousarray(ck.reshape(cols, 16).T), (N_CORES, 1))
            for ck in flat.reshape(n_chunks, per_chunk)
        ]
        m["idxs"] = np.ascontiguousarray(np.stack(chunks, axis=1))
        in_maps.append(m)

    nc = _get_module(("masked", n_tiles, pad_idx), _build_module_masked,
                     n_tiles, pad_idx)
    res = run_bass_kernel_spmd(nc, in_maps, core_ids=list(range(N_CORES)),
                               trace=_trace)
    out = np.concatenate([res.results[c]["h"][:npc] for c in range(N_CORES)],
                         axis=0)
    if _trace:
        return out, res
    return out
